# revision 45
# baseline (speedup 1.0000x reference)
"""Trainium2 Bass kernel for masked cosine-similarity attention scores.

Problem: nn_MultiHeadedAttention_2 (sparse_attention, memory-bound)
  query [16, 1, 1024] f32, key [16, 8192, 1024] f32, mask [16, 8192] int32
  out   [16, 16, 8192] f32 = relu(cos_sim_per_head(q, k) masked) / Lk

Math (per batch b, head h, key position l):
  num[h,l] = sum_d q[h,d] * k[l, h*64+d]
  kn[h,l]  = ||k[l, h*64:(h+1)*64]||
  p        = relu(num / (qn[h] * kn)) * mask[l] / Lk
           = relu(sum_d qtilde[h,d] * k[...]) * exp(-0.5*ln(kn^2) + lnm[l])
  where qtilde = q / (qn * Lk) is folded on the host (input prep) and
  lnm[l] = 0 if mask else -1e30 (exp(...-1e30) == 0 -> exact masked zero).
  The reference's EPS=1e-8 guard on qn*kn is unreachable for randn inputs
  (qn, kn ~ sqrt(64)), so it is not emulated.

Sharding: data-parallel over batch B=16 -> 2 batches per core x 8 cores.

Final design (HW slope-measured 306.8us/core vs 85.3ms baseline print;
cost model 300.3us):
  - Keys ship from the host pre-cast to bf16 (numerically identical to
    the previous in-DMA SWDGE cast; halves HBM + H2D, frees Pool).
  - Key row l lives at (partition p, tile t) with l = p*T + t, so each
    partition streams one contiguous DRAM chunk per 8-tile group over
    the two HWDGE rings (alternating nc.sync / nc.scalar).
  - HW measurement showed cross-engine pipelining does not overlap on
    this walrus/axon stack (per-unit time ~= SUM of engine busy times;
    DVE+Pool actively contend via shared SBUF ports), so the bulk math
    runs single-engine on DVE at bf16 2x: k*qb mult and k*k square into
    one shared [P,(2 TG H),64] tile, 5 pairwise fold levels (each one
    instruction covering both paths, fresh tiles — in-place folds stall
    the DVE stream), and ONE merged [P,512,2]->[P,512] f32 reduce.
    ACT only does Ln + Exp(scale=-0.5) = 1/kn, Pool only the mask
    broadcast-multiply; relu*rk lands back on DVE in bf16 and each
    unit's output block is DMA'd out immediately (no serial tail DMA).
  - PE/PSUM unused: output is written [p, (b,t,h)]-packed bf16 and
    untangled/upcast on the host; mask prep is a pure reshape.

An optional repeat=R wraps the whole per-core body in a hardware loop
(tc.For_i) so test.py can measure true HW kernel time by slope:
(wall(R_hi) - wall(R_lo)) / (R_hi - R_lo) cancels the ~80ms axon RPC
dispatch constant that dominates single-shot wall time.

Self-contained: only imports the platform libs from /opt/trn_rl_repo.
"""

import os
import sys

sys.path.insert(0, "/opt/trn_rl_repo")

import numpy as np

import concourse.bass as bass
import concourse.mybir as mybir
from concourse.tile import TileContext
from concourse.bass_utils import run_bass_kernel_spmd  # noqa: F401 (platform entry)

# Keep the number of active DMA completion-sem lanes low: the kernel-tail
# Drain waits on every active proc's semaphore and walrus rejects
# instructions with too many sync waits. Lanes are bookkeeping sems (FIFO
# per ring), not HW queues, so this does not serialize the transfers.
import concourse.tile_sem_assignment as _tsa

_tsa.NUM_HWDGE_SEMS = 2
_tsa.NUM_SWDGE_GLOBAL_SEMS = 2

# The walrus build in this environment accepts at most ONE sync wait per
# instruction. Tile's scheduler can emit several (cross-engine RAW + WAR +
# DMA-lane waits). Splitting the extra waits into standalone EventSemaphore
# instructions on the same engine is semantically identical: the engine's
# sequencer executes them in order immediately before the instruction.
import orjson as _orjson


def _split_multi_waits(bir_bytes: bytes) -> bytes:
    m = _orjson.loads(bir_bytes)
    changed = False
    for fn in m.get("functions", []):
        for bb in fn.get("blocks", []):
            insts = bb.get("instructions")
            if not insts:
                continue
            out_list = []
            for inst in insts:
                si = inst.get("sync_info")
                waits = (si or {}).get("on_wait") or []
                if len(waits) > 1:
                    changed = True
                    for k, w in enumerate(waits[:-1]):
                        out_list.append(
                            {
                                "debug": inst.get("debug", 0),
                                "engine": inst["engine"],
                                "ins": [],
                                "name": f"{inst['name']}_wsplit{k}",
                                "opcode": "EventSemaphore",
                                "outs": [],
                                "sync_info": {"on_update": [], "on_wait": [w]},
                            }
                        )
                    si["on_wait"] = [waits[-1]]
                out_list.append(inst)
            bb["instructions"] = out_list
    return _orjson.dumps(m) if changed else bir_bytes


_orig_to_json_bytes = bass.Bass.to_json_bytes


def _patched_to_json_bytes(self, *a, **kw):
    return _split_multi_waits(_orig_to_json_bytes(self, *a, **kw))


bass.Bass.to_json_bytes = _patched_to_json_bytes

F32 = mybir.dt.float32
BF16 = mybir.dt.bfloat16
I32 = mybir.dt.int32
Alu = mybir.AluOpType
Act = mybir.ActivationFunctionType
AX = mybir.AxisListType

H = 16      # heads
DK = 64     # head dim
DM = 1024   # d_model
P = 128     # SBUF partitions
N_CORES = 8

# Compute dtype for the streamed key data ("f32" or "bf16").
PRECISION = os.environ.get("COSSIM_PRECISION", "bf16")
# Engine-assignment mode. HW slope measurements:
#   "split"  (v6: folds spread DVE/Pool, shared okt tile): 455us/iter —
#            engines effectively serialized (tight bufs=2 WAR coupling +
#            multi-wait splits).
#   "alldve" (v7: square + all folds on DVE):              327us/iter —
#            matches the DVE-busy cost model within 3%.
#   "split2" (v8: k^2 path on ACT+Pool, separate per-path reduces, deep
#            handoff buffers, ~3 DVE waits/unit): candidate.
MODE = os.environ.get("COSSIM_MODE", "alldve")
ALLDVE = MODE in ("alldve", "sqact")
SQACT = MODE == "sqact"   # alldve structure but the square runs on ACT


def build_nc(n_batch: int, lk: int, precision: str = PRECISION,
             repeat: int = 1) -> bass.Bass:
    """Build the per-core Bass program.

    Per-core DRAM I/O (T = lk//128 tiles; key row l lives at (p, t) with
    l = p*T + t so each partition's group slice is DRAM-contiguous):
      key   [n_batch, lk, 1024] f32   (shard of the key tensor)
      qb    [n_batch, 128, 1024] cdt  (host-broadcast qtilde rows)
      maskr [n_batch, 128, T] i32     (mask.reshape(128, T))
      out   [n_batch, 128, T*16] f32  (out[b, p, t*16+h] = p[b, h, p*T+t])
    """
    assert n_batch == 2, "kernel assumes a batch pair per core"
    cdt = BF16 if precision == "bf16" else F32
    ntiles = lk // P            # 128-key subtiles per batch
    # subtiles per DMA group / pipeline unit (16 halves per-op overheads;
    # fits SBUF only with the bufs=1 reversed-stage emission)
    TG = min(int(os.environ.get("COSSIM_TG", "8")), ntiles)
    ngroups = ntiles // TG
    # post-fold segment width: FW=4 -> 4 pairwise bf16 fold levels before
    # the f32 reduce (HW slope 326.0us/iter vs 327.6 at FW=8; rel err
    # 5.8e-3 vs 5.3e-3, both far under the 2e-2 gate)
    FW = int(os.environ.get("COSSIM_FW", "2"))

    nc = bass.Bass()
    # the host ships the key shard already cast to the compute dtype
    key_in = nc.declare_dram_parameter("key", [n_batch, lk, DM], cdt, isOutput=False)
    qb_in = nc.declare_dram_parameter("qb", [n_batch, P, DM], cdt, isOutput=False)
    mask_in = nc.declare_dram_parameter(
        "maskr", [n_batch, P, ntiles], I32, isOutput=False
    )
    # device layout: out[p, b*ntiles*16 + t*16 + h]; host untangles and
    # upcasts (bf16 out: p values round at 2^-9 rel ~ 2e-7 abs, far under
    # the 2e-2 gate; halves the out DMA and gives the final mult 2x)
    out = nc.declare_dram_parameter("out", [P, n_batch * ntiles * H], cdt,
                                    isOutput=True)
    out_flat = out

    with TileContext(nc) as tc:
        with (
            tc.tile_pool(name="const", bufs=1) as cpool,
            tc.tile_pool(name="kbig", bufs=3) as kpool,
            tc.tile_pool(name="work", bufs=2) as wpool,
            tc.tile_pool(name="small", bufs=3) as spool,
            tc.tile_pool(name="outp", bufs=1) as opool,
        ):
            # constants, staged through DVE so consumers only dep on DVE
            qbs, maskfs = [], []
            for b in range(n_batch):
                qb_r = cpool.tile([P, DM], cdt, name=f"qbr{b}")
                nc.gpsimd.dma_start(out=qb_r[:], in_=qb_in[b])
                qb = cpool.tile([P, DM], cdt, name=f"qbs{b}")
                nc.vector.tensor_copy(qb[:], qb_r[:])
                qbs.append(qb)
                maskt = cpool.tile([P, ntiles], I32, name=f"maskt{b}")
                nc.gpsimd.dma_start(out=maskt[:], in_=mask_in[b])
                maskf = cpool.tile([P, ntiles], F32, name=f"maskf{b}")
                nc.vector.tensor_copy(maskf[:], maskt[:])
                maskfs.append(maskf)

            # key DRAM view: l = p*T + t  ->  [p, t, c]
            key_v = [key_in[b].rearrange("(p t) c -> p t c", p=P)
                     for b in range(n_batch)]

            # unit u = (b, g): one TG-tile group of one batch
            units = [(u % n_batch, u // n_batch) for u in range(n_batch * ngroups)]
            nunits = len(units)
            DMA_LEAD = 2   # kt DMA issued this many rounds before stage A

            def body(_iv=None):
                outacc = opool.tile([P, n_batch * ntiles * H], cdt,
                                    name="outacc", tag="outacc")
                kts, prods, sqs, okts, gns2s = {}, {}, {}, {}, {}

                def issue_dma(u):
                    b, g = units[u]
                    kt = kpool.tile([P, TG * DM], cdt, name="kt", tag="kt")
                    src = key_v[b][:, g * TG : (g + 1) * TG, :]
                    dst = kt.rearrange("p (t c) -> p t c", c=DM)
                    # TRN2 has two physical HWDGE rings (qSPDynamicHW,
                    # qActDynamicHW), FIFO per issuing engine — alternate the
                    # issuing engine so both rings stream keys in parallel.
                    eng = nc.sync if (u % 2 == 0) else nc.scalar
                    eng.dma_start(out=dst, in_=src)  # HWDGE, no cast
                    kts[u] = kt

                def fold_chain(eng, tagp, cur, w, dst4):
                    """Pairwise fold `cur` (seg width w) down to FW on `eng`;
                    the final level writes into dst4 [P,TG,H,FW]."""
                    while w > 2 * FW:
                        nxt = wpool.tile([P, TG * H * (w // 2)], cdt,
                                         name=f"{tagp}{w // 2}",
                                         tag=f"{tagp}{w // 2}")
                        nv = nxt.rearrange("p (t h d) -> p t h d", h=H,
                                           d=w // 2)
                        eng.tensor_tensor(
                            nv, cur[:, :, :, 0 : w // 2],
                            cur[:, :, :, w // 2 : w], Alu.add)
                        cur, w = nv, w // 2
                    eng.tensor_tensor(
                        dst4, cur[:, :, :, 0:FW], cur[:, :, :, FW : 2 * FW],
                        Alu.add)

                def stage_a(u):
                    """mult + square, both DVE bf16 2x in alldve, written into
                    ONE shared psq tile [P, (2 TG H), 64] so every fold level
                    is a single instruction covering both paths."""
                    b, g = units[u]
                    ks = kts.pop(u)
                    if ALLDVE:
                        psq = wpool.tile([P, 2 * TG * DM], cdt, name="psq",
                                         tag="psq")
                        nc.vector.tensor_tensor(
                            psq[:, 0 : TG * DM].rearrange("p (t c) -> p t c",
                                                          c=DM),
                            ks.rearrange("p (t c) -> p t c", c=DM),
                            qbs[b][:].rearrange("p (o c) -> p o c", o=1)
                            .broadcast_to([P, TG, DM]),
                            Alu.mult,
                        )
                        if SQACT:
                            nc.scalar.activation(
                                psq[:, TG * DM : 2 * TG * DM], ks[:],
                                Act.Square)
                        else:
                            nc.vector.tensor_tensor(
                                psq[:, TG * DM : 2 * TG * DM], ks[:], ks[:],
                                Alu.mult)
                        prods[u] = psq
                        return
                    prod = wpool.tile([P, TG * DM], cdt, name="prod", tag="prod")
                    nc.vector.tensor_tensor(
                        prod.rearrange("p (t c) -> p t c", c=DM),
                        ks.rearrange("p (t c) -> p t c", c=DM),
                        qbs[b][:].rearrange("p (o c) -> p o c", o=1)
                        .broadcast_to([P, TG, DM]),
                        Alu.mult,
                    )
                    sq = wpool.tile([P, TG * DM], cdt, name="sq", tag="sq")
                    nc.scalar.activation(sq[:], ks[:], Act.Square)
                    prods[u], sqs[u] = prod, sq

                def stage_b(u):
                    """alldve: single half-fold covering both paths (DVE).
                    split2: the ENTIRE k^2 fold chain on Pool into its own
                    oks tile (single writer; DVE touches it only at red_s)."""
                    if ALLDVE:
                        psq = prods.pop(u)
                        w = DK // 2
                        cur = psq.rearrange("p (s d) -> p s d", d=DK)
                        nxt = wpool.tile([P, 2 * TG * H * w], cdt,
                                         name=f"fb{w}", tag=f"fb{w}")
                        nc.vector.tensor_tensor(
                            nxt.rearrange("p (s d) -> p s d", d=w),
                            cur[:, :, 0:w], cur[:, :, w : 2 * w],
                            Alu.add)
                        okts[u] = (nxt, w)
                    else:
                        sq = sqs.pop(u)
                        oks = wpool.tile([P, TG * H * FW], cdt, name="oks",
                                         tag="oks")
                        fold_chain(nc.gpsimd, "ps",
                                   sq.rearrange("p (t h d) -> p t h d", h=H,
                                                d=DK),
                                   DK, oks.rearrange("p (t h d) -> p t h d",
                                                     h=H, d=FW))
                        okts[u] = oks

                def stage_c(u):
                    """alldve: remaining both-path folds + ONE reduce (DVE).
                    split2: num fold chain + per-path reduces (DVE)."""
                    if ALLDVE:
                        cur, w = okts.pop(u)
                        cur = cur.rearrange("p (s d) -> p s d", d=w)
                        while w > 2 * FW:
                            nxt = wpool.tile([P, 2 * TG * H * (w // 2)], cdt,
                                             name=f"fb{w // 2}",
                                             tag=f"fb{w // 2}")
                            nv = nxt.rearrange("p (s d) -> p s d", d=w // 2)
                            nc.vector.tensor_tensor(
                                nv, cur[:, :, 0 : w // 2],
                                cur[:, :, w // 2 : w], Alu.add)
                            cur, w = nv, w // 2
                        okt = wpool.tile([P, 2 * TG * H * FW], cdt, name="okt",
                                         tag="okt")
                        o4 = okt.rearrange("p (s d) -> p s d", d=FW)
                        nc.vector.tensor_tensor(
                            o4, cur[:, :, 0:FW], cur[:, :, FW : 2 * FW],
                            Alu.add)
                        gns2 = spool.tile([P, 2 * TG * H], F32, name="gns2",
                                          tag="gns2")
                        nc.vector.reduce_sum(gns2[:], o4, axis=AX.X)
                        gns2s[u] = gns2
                    else:
                        prod = prods.pop(u)
                        okn = wpool.tile([P, TG * H * FW], cdt, name="okn",
                                         tag="okn")
                        on4 = okn.rearrange("p (t h d) -> p t h d", h=H, d=FW)
                        fold_chain(nc.vector, "gn",
                                   prod.rearrange("p (t h d) -> p t h d", h=H,
                                                  d=DK),
                                   DK, on4)
                        gnum = spool.tile([P, TG * H], F32, name="gnum",
                                          tag="gnum")
                        nc.vector.reduce_sum(
                            gnum.rearrange("p (t h) -> p t h", h=H), on4,
                            axis=AX.X)
                        gs2 = spool.tile([P, TG * H], F32, name="gs2",
                                         tag="gs2")
                        nc.vector.reduce_sum(
                            gs2.rearrange("p (t h) -> p t h", h=H),
                            okts.pop(u).rearrange("p (t h d) -> p t h d", h=H,
                                                  d=FW),
                            axis=AX.X)
                        gns2s[u] = (gnum, gs2)

                def stage_d(u):
                    """ln/exp (ACT), mask mult (Pool), relu*rk (DVE)."""
                    b, g = units[u]
                    if ALLDVE:
                        # gns2 layout [ (x t h) ]: x=0 -> num, x=1 -> s2
                        g2 = gns2s.pop(u)
                        num = g2[:, 0 : TG * H].rearrange("p (t h) -> p t h",
                                                          h=H)
                        s2 = g2[:, TG * H : 2 * TG * H].rearrange(
                            "p (t h) -> p t h", h=H)
                    else:
                        gnum, gs2 = gns2s.pop(u)
                        num = gnum.rearrange("p (t h) -> p t h", h=H)
                        s2 = gs2.rearrange("p (t h) -> p t h", h=H)
                    lns = spool.tile([P, TG * H], F32, name="lns", tag="lns")
                    nc.scalar.activation(
                        lns.rearrange("p (t h) -> p t h", h=H), s2, Act.Ln)
                    rk = spool.tile([P, TG * H], F32, name="rk", tag="rk")
                    # rk = exp(-0.5 * ln(s2)) = 1/kn
                    nc.scalar.activation(rk[:], lns[:], Act.Exp, scale=-0.5)
                    # rkm = rk * mask (mask broadcast over heads)
                    rkm = spool.tile([P, TG * H], F32, name="rkm", tag="rkm")
                    mcol = maskfs[b][:, g * TG : (g + 1) * TG]
                    nc.gpsimd.tensor_tensor(
                        rkm.rearrange("p (t h) -> p t h", h=H),
                        rk.rearrange("p (t h) -> p t h", h=H),
                        mcol.rearrange("p (t o) -> p t o", o=1).broadcast_to(
                            [P, TG, H]),
                        Alu.mult,
                    )
                    # outacc block = max(num, 0) * rkm (DVE; walrus has no
                    # Pool lowering for scalar_tensor_tensor)
                    o0 = (b * ntiles + g * TG) * H
                    nc.vector.scalar_tensor_tensor(
                        outacc[:, o0 : o0 + TG * H].rearrange(
                            "p (t h) -> p t h", h=H),
                        num,
                        0.0,
                        rkm.rearrange("p (t h) -> p t h", h=H),
                        Alu.max,
                        Alu.mult,
                    )
                    # stream this unit's block out now (overlaps compute,
                    # empties the DMA queue before the loop back-edge drain)
                    nc.sync.dma_start(out=out_flat[:, o0 : o0 + TG * H],
                                      in_=outacc[:, o0 : o0 + TG * H])

                for u in range(min(DMA_LEAD, nunits)):
                    issue_dma(u)
                for r in range(nunits + 3):
                    if r + DMA_LEAD < nunits:
                        issue_dma(r + DMA_LEAD)
                    if r < nunits:
                        stage_a(r)
                    if 0 <= r - 1 < nunits:
                        stage_b(r - 1)
                    if 0 <= r - 2 < nunits:
                        stage_c(r - 2)
                    if 0 <= r - 3 < nunits:
                        stage_d(r - 3)

            if repeat == 1:
                body()
            else:
                with tc.For_i(0, repeat, 1) as iv:
                    body(iv)
    return nc


_NC_CACHE: dict = {}


def _get_nc(n_batch, lk, precision=PRECISION, repeat=1):
    key = (n_batch, lk, precision, repeat)
    if key not in _NC_CACHE:
        _NC_CACHE[key] = build_nc(n_batch, lk, precision, repeat)
    return _NC_CACHE[key]


def _host_prep(query, key, mask, precision=PRECISION):
    """Host-side input prep (layout & folding of scalars into qtilde)."""
    B, lk, dm = key.shape
    assert dm == DM
    cdt_np = mybir.dt.np(BF16 if precision == "bf16" else F32)

    q = query.reshape(B, H, DK).astype(np.float64)
    qn = np.sqrt((q * q).sum(-1))  # [B, H]
    qt = q / (qn[:, :, None] * float(lk))  # qtilde [B, H, DK]
    qb = np.ascontiguousarray(
        np.broadcast_to(qt.reshape(B, 1, DM), (B, P, DM))
    ).astype(cdt_np)

    ntiles = lk // P
    # key row l lives at (p, t) with l = p*ntiles + t -> plain reshape
    maskr = np.ascontiguousarray(mask.reshape(B, P, ntiles)).astype(np.int32)
    return qb, maskr


def prep_inputs(query, key, mask, n_cores=N_CORES, precision=PRECISION):
    """Shard into per-core input maps (for CoreSim / run_bass_kernel_spmd)."""
    B = key.shape[0]
    nb = B // n_cores
    qb, maskr = _host_prep(query, key, mask, precision)
    cdt_np = mybir.dt.np(BF16 if precision == "bf16" else F32)
    keyc = np.ascontiguousarray(key).astype(cdt_np)
    in_maps = []
    for c in range(n_cores):
        sl = slice(c * nb, (c + 1) * nb)
        in_maps.append(
            {
                "key": keyc[sl],
                "qb": qb[sl],
                "maskr": maskr[sl],
            }
        )
    return in_maps


def unshuffle_out(out_raw, B, lk, nb=2):
    """Per-core [128, nb*T*16] device layout (stacked over cores on axis 0)
    -> [B, H, lk] with l = p*T + t."""
    ntiles = lk // P
    a = out_raw.reshape(B // nb, P, nb, ntiles, H)  # [core, p, b, t, h]
    return np.ascontiguousarray(
        a.transpose(0, 2, 4, 1, 3).astype(np.float32)).reshape(B, H, lk)


class _Runner:
    """Cached PJRT executable for one built Bass program.

    Mirrors bass2jax.run_bass_via_pjrt but jits ONCE, and feeds the
    global (unsharded) arrays directly: shard_map splits axis 0 across
    the 8 cores, which is exactly the per-core batch shard.
    """

    def __init__(self, nc, n_cores):
        import jax
        from jax.sharding import Mesh, PartitionSpec
        from jax.experimental.shard_map import shard_map
        from concourse import bass2jax as b2j

        b2j.install_neuronx_cc_hook()
        self.jax = jax
        self.n_cores = n_cores
        part_name = (
            nc.partition_id_tensor.name if nc.partition_id_tensor else None
        )
        in_names, out_names, out_avals, zero_outs = [], [], [], []
        for alloc in nc.m.functions[0].allocations:
            if not isinstance(alloc, mybir.MemoryLocationSet):
                continue
            name = alloc.memorylocations[0].name
            if alloc.kind == "ExternalInput":
                if name != part_name:
                    in_names.append(name)
            elif alloc.kind == "ExternalOutput":
                out_names.append(name)
                shape = tuple(alloc.tensor_shape)
                dtype = mybir.dt.np(alloc.dtype)
                out_avals.append(jax.core.ShapedArray(shape, dtype))
                zero_outs.append(np.zeros(shape, dtype))
        self.in_names, self.out_names = in_names, out_names
        self.out_avals, self.zero_outs = out_avals, zero_outs
        n_params, n_outs = len(in_names), len(out_names)

        bind_in_names = in_names + out_names
        if part_name is not None:
            bind_in_names = bind_in_names + [part_name]

        def _body(*args):
            operands = list(args)
            if part_name is not None:
                operands.append(b2j.partition_id_tensor())
            outs = b2j._bass_exec_p.bind(
                *operands,
                out_avals=tuple(out_avals),
                in_names=tuple(bind_in_names),
                out_names=tuple(out_names),
                lowering_input_output_aliases=(),
                sim_require_finite=True,
                sim_require_nnan=True,
                nc=nc,
            )
            return tuple(outs)

        devices = jax.devices()[:n_cores]
        self.mesh = Mesh(np.asarray(devices), ("core",))
        in_specs = (PartitionSpec("core"),) * (n_params + n_outs)
        out_specs = (PartitionSpec("core"),) * n_outs
        self.fn = jax.jit(
            shard_map(
                _body,
                mesh=self.mesh,
                in_specs=in_specs,
                out_specs=out_specs,
                check_rep=False,
            ),
            donate_argnums=tuple(range(n_params, n_params + n_outs)),
            keep_unused=True,
        )

    def global_args(self, global_ins: dict):
        args = [global_ins[name] for name in self.in_names]
        args += [
            np.zeros((self.n_cores * z.shape[0], *z.shape[1:]), z.dtype)
            for z in self.zero_outs
        ]
        return args

    def __call__(self, global_ins: dict):
        out_arrs = self.fn(*self.global_args(global_ins))
        return {
            name: np.asarray(out_arrs[i]) for i, name in enumerate(self.out_names)
        }


_RUNNER_CACHE: dict = {}


def _get_runner(n_batch, lk, precision=PRECISION, repeat=1):
    key = (n_batch, lk, precision, repeat)
    if key not in _RUNNER_CACHE:
        nc = _get_nc(n_batch, lk, precision, repeat)
        if not nc.is_finalized():
            nc.finalize()
        _RUNNER_CACHE[key] = _Runner(nc, N_CORES)
    return _RUNNER_CACHE[key]


def global_inputs(query, key, mask, precision=PRECISION):
    """Host prep producing the UNSHARDED arrays fed to shard_map (axis 0
    splits evenly across the 8 cores == batch sharding). The key shard is
    shipped pre-cast to the compute dtype (halves H2D + HBM traffic; the
    device compute is identical to the previous in-DMA f32->bf16 cast)."""
    qb, maskr = _host_prep(query, key, mask, precision)
    cdt_np = mybir.dt.np(BF16 if precision == "bf16" else F32)
    return {"key": np.ascontiguousarray(key).astype(cdt_np), "qb": qb,
            "maskr": maskr}


def kernel(query, key, mask):
    B, lk, _ = key.shape
    nb = B // N_CORES
    runner = _get_runner(nb, lk)
    gins = global_inputs(query, key, mask)
    out = runner(gins)["out"]  # [B, 128, T*16] concat over cores on axis 0
    return unshuffle_out(out, B, lk)


if __name__ == "__main__":
    # smoke test at reduced size
    rng = np.random.default_rng(0)
    B, lk = 16, 1024
    query = rng.standard_normal((B, 1, DM), dtype=np.float32)
    key = rng.standard_normal((B, lk, DM), dtype=np.float32)
    mask = rng.integers(0, 2, (B, lk)).astype(np.int32)
    out = kernel(query, key, mask)
    print("out", out.shape, out.dtype, float(np.abs(out).max()))
